# revision 2
# baseline (speedup 1.0000x reference)
"""GQA attention block (dense_transformer) on 8 trn2 cores.

Sharding: tensor-parallel by kv-group. Core c owns kv-group c = 8 query
heads + 1 k + 1 v head (640 rows of W_qkv) and the matching 512 columns of
W_dense. hidden_states is replicated (passed transposed, bf16). Each core
returns a partial [4096, 2048] dense output; the host sums the 8 partials.

On-core layout is feature-major ([feature, position]) throughout:
  qkvT = Wg @ hsT via PE (bf16), RoPE on DVE with a/b half-split weight
  permutation, scoresT[k,q] per k-tile (alibi enters as the per-partition
  ACT bias of the exp), PV with position-major V (PE transpose) augmented
  with a ones column so the softmax denominator falls out of the same
  matmul, 1/l via Ln+Exp on ACT, broadcast along partitions via a f32r
  ones-matmul, dense partial straight out of PSUM to DRAM.
"""
import numpy as np
import ml_dtypes
from contextlib import ExitStack

import bass_rust
import concourse.bass as bass
import concourse.mybir as mybir
from concourse import tile
from concourse.bass_utils import run_bass_kernel_spmd
from concourse.vector_clock import ScopedClock

dt = mybir.dt
bf16 = ml_dtypes.bfloat16

B, S, HID = 2, 1024, 4096
NKV, G, HD = 8, 8, 64
NPOS = B * S
INV = 0.125
NCORES = 8

# ---------------------------------------------------------------------------
# walrus in this container takes at most ONE sync-wait per instruction; Tile
# attaches several (tail drain especially). Split extras onto same-engine nops.
_orig_exit = tile.TileContext.__exit__


def _split_waits(nc):
    for bb in nc.m.functions[0].blocks:
        out, extra = [], 0
        for inst in bb.instructions:
            si = inst.sync_info
            if si is not None and len(si.on_wait) > 1:
                waits = list(si.on_wait)
                for w in waits[:-1]:
                    nop = mybir.InstNoOp(name=f"I-wsplit-{nc.next_id()}")
                    nop.engine = inst.engine
                    nop.sync_info = bass_rust.SyncInfo(on_wait=[w], on_update=[])
                    nc.register_instruction(nop, overwrite=True)
                    out.append(nop)
                    extra += 1
                inst.sync_info = bass_rust.SyncInfo(
                    on_wait=[waits[-1]], on_update=list(si.on_update)
                )
            out.append(inst)
        if extra:
            bb.instructions = out


def _patched_exit(self, exc_type, exc_val, exc_tb):
    r = _orig_exit(self, exc_type, exc_val, exc_tb)
    _split_waits(self.nc)
    return r


tile.TileContext.__exit__ = _patched_exit
# ---------------------------------------------------------------------------

_CACHED_NC = None


def build_program():
    global _CACHED_NC
    if _CACHED_NC is not None:
        return _CACHED_NC
    nc = bass.Bass()
    hst_d = nc.declare_dram_parameter("hst", [32, 128, NPOS], dt.bfloat16, isOutput=False)
    wq_d = nc.declare_dram_parameter("wq", [32, 128, 640], dt.bfloat16, isOutput=False)
    wd_d = nc.declare_dram_parameter("wd", [4, 128, 4096], dt.bfloat16, isOutput=False)
    cst_d = nc.declare_dram_parameter("cst", [128, 2048], dt.bfloat16, isOutput=False)
    msk_d = nc.declare_dram_parameter("msk", [128, 2048], dt.float32, isOutput=False)
    ab_d = nc.declare_dram_parameter("ab", [128, 128], dt.float32, isOutput=False)
    idn_d = nc.declare_dram_parameter("idn", [128, 128], dt.bfloat16, isOutput=False)
    outp_d = nc.declare_dram_parameter("outp", [32, 128, NPOS], dt.float32, isOutput=True)
    dbg_qp = nc.declare_dram_parameter("dbg_qp", [2, 6, 128, 1024], dt.bfloat16, isOutput=True)
    dbg_ct = nc.declare_dram_parameter("dbg_ct", [2, 4, 128, 1024], dt.bfloat16, isOutput=True)
    dbg_e = nc.declare_dram_parameter("dbg_e", [2, 128, 1024], dt.bfloat16, isOutput=True)
    dbg_rb = nc.declare_dram_parameter("dbg_rb", [2, 64, 1024], dt.float32, isOutput=True)
    dbg_l = nc.declare_dram_parameter("dbg_l", [2, 2, 1024], dt.float32, isOutput=True)
    dbg_cps = nc.declare_dram_parameter("dbg_cps", [2, 65, 1024], dt.float32, isOutput=True)

    AF = mybir.ActivationFunctionType

    with ExitStack() as ctx:
        tc = ctx.enter_context(tile.TileContext(nc))
        cpool = ctx.enter_context(tc.tile_pool(name="const", bufs=1))
        wq_sb = cpool.tile([128, 32 * 640], dt.bfloat16)
        for k in range(32):
            nc.sync.dma_start(wq_sb[:, k * 640:(k + 1) * 640], wq_d[k])
        wd_sb = cpool.tile([128, 4 * 4096], dt.bfloat16)
        for kt in range(4):
            nc.sync.dma_start(wd_sb[:, kt * 4096:(kt + 1) * 4096], wd_d[kt])
        cst_sb = cpool.tile([128, 2048], dt.bfloat16)
        nc.sync.dma_start(cst_sb[:], cst_d[:])
        msk_sb = cpool.tile([128, 2048], dt.float32)
        nc.sync.dma_start(msk_sb[:], msk_d[:])
        ab_sb = cpool.tile([128, 128], dt.float32)
        nc.sync.dma_start(ab_sb[:], ab_d[:])
        idn_sb = cpool.tile([128, 128], dt.bfloat16)
        nc.sync.dma_start(idn_sb[:], idn_d[:])
        onesf = cpool.tile([1, 64], dt.float32)
        nc.vector.memset(onesf[:], 1.0)
        ones_r = cpool.tile([1, 64], dt.float32r)
        nc.scalar.copy(ones_r[:], onesf[:])

        hs_pool = ctx.enter_context(tc.tile_pool(name="hs", bufs=4))
        raw_pool = ctx.enter_context(tc.tile_pool(name="raw", bufs=2))
        tmp_pool = ctx.enter_context(tc.tile_pool(name="tmp", bufs=2))
        pk_pool = ctx.enter_context(tc.tile_pool(name="pk", bufs=2))
        qp_pool = ctx.enter_context(tc.tile_pool(name="qp", bufs=1))
        kv_pool = ctx.enter_context(tc.tile_pool(name="kv", bufs=1))
        va_pool = ctx.enter_context(tc.tile_pool(name="va", bufs=1))
        exp_pool = ctx.enter_context(tc.tile_pool(name="exp", bufs=1))
        l_pool = ctx.enter_context(tc.tile_pool(name="l", bufs=1))
        rb_pool = ctx.enter_context(tc.tile_pool(name="rb", bufs=2))
        ctx_pool = ctx.enter_context(tc.tile_pool(name="ctx", bufs=1))

        for b in range(2):
            # ---------------- QKV projection + RoPE ----------------
            qp = [qp_pool.tile([128, 1024], dt.bfloat16, tag=f"qp{p}", name=f"qp{p}") for p in range(4)]
            kdup = kv_pool.tile([128, 1024], dt.bfloat16, tag="kdup")
            vt = kv_pool.tile([64, 1024], dt.bfloat16, tag="vt")
            with tc.tile_pool(name="qkvps", bufs=1, space="PSUM") as qkv_psum:
                for n in range(2):
                    pcol = b * 1024 + n * 512
                    ncol = slice(n * 512, n * 512 + 512)
                    ps = [qkv_psum.tile([128, 512], dt.float32, tag=f"qkv{m}", name=f"qkv{m}")
                          for m in range(5)]
                    for k in range(32):
                        hs_t = hs_pool.tile([128, 512], dt.bfloat16)
                        nc.sync.dma_start(hs_t[:], hst_d[k][:, pcol:pcol + 512])
                        for m in range(5):
                            nc.tensor.matmul(
                                ps[m][:],
                                wq_sb[:, k * 640 + m * 128: k * 640 + (m + 1) * 128],
                                hs_t[:],
                                start=(k == 0), stop=(k == 31),
                            )
                    raw = [raw_pool.tile([128, 512], dt.bfloat16, tag=f"raw{m}", name=f"raw{m}")
                           for m in range(5)]
                    for m in range(5):
                        nc.scalar.copy(raw[m][:], ps[m][:])
                    Cs = cst_sb[:, n * 512:(n + 1) * 512]
                    Ss = cst_sb[:, 1024 + n * 512: 1024 + (n + 1) * 512]
                    for grp in range(2):
                        A, Bb = raw[grp * 2], raw[grp * 2 + 1]
                        P1 = tmp_pool.tile([128, 512], dt.bfloat16, tag="P1")
                        P2 = tmp_pool.tile([128, 512], dt.bfloat16, tag="P2")
                        P3 = tmp_pool.tile([128, 512], dt.bfloat16, tag="P3")
                        P4 = tmp_pool.tile([128, 512], dt.bfloat16, tag="P4")
                        nc.vector.tensor_mul(P1[:], A[:], Cs)
                        nc.vector.tensor_mul(P2[:], Bb[:], Ss)
                        nc.vector.tensor_mul(P3[:], Bb[:], Cs)
                        nc.vector.tensor_mul(P4[:], A[:], Ss)
                        for i in range(4):
                            h = grp * 4 + i
                            pr, sub = h // 2, h % 2
                            sl = slice(32 * i, 32 * i + 32)
                            nc.vector.tensor_sub(
                                qp[pr][sub * 64: sub * 64 + 32, ncol], P1[sl, :], P2[sl, :])
                            nc.vector.tensor_add(
                                qp[pr][sub * 64 + 32: sub * 64 + 64, ncol], P3[sl, :], P4[sl, :])
                    kvr = raw[4]
                    pk1 = pk_pool.tile([32, 512], dt.bfloat16, tag="pk1")
                    pk2 = pk_pool.tile([32, 512], dt.bfloat16, tag="pk2")
                    pk3 = pk_pool.tile([32, 512], dt.bfloat16, tag="pk3")
                    pk4 = pk_pool.tile([32, 512], dt.bfloat16, tag="pk4")
                    nc.vector.tensor_mul(pk1[:], kvr[0:32, :], Cs[0:32, :])
                    nc.vector.tensor_mul(pk2[:], kvr[32:64, :], Ss[32:64, :])
                    nc.vector.tensor_mul(pk3[:], kvr[32:64, :], Cs[32:64, :])
                    nc.vector.tensor_mul(pk4[:], kvr[0:32, :], Ss[0:32, :])
                    nc.vector.tensor_sub(kdup[0:32, ncol], pk1[:], pk2[:])
                    nc.vector.tensor_add(kdup[32:64, ncol], pk3[:], pk4[:])
                    nc.vector.tensor_copy(kdup[64:128, ncol], kdup[0:64, ncol])
                    nc.vector.tensor_copy(vt[:, ncol], kvr[64:128, :])

            # ---------------- V transpose + ones column ----------------
            va = va_pool.tile([128, 8 * 72], dt.bfloat16, tag="va")
            with tc.tile_pool(name="vtps", bufs=2, space="PSUM") as vt_psum:
                for ki in range(8):
                    vps = vt_psum.tile([128, 64], dt.bfloat16, tag="vps")
                    nc.tensor.transpose(vps[:], vt[0:64, ki * 128:(ki + 1) * 128],
                                        idn_sb[0:64, 0:64])
                    nc.scalar.copy(va[:, ki * 72: ki * 72 + 64], vps[:])
                    nc.vector.memset(va[:, ki * 72 + 64: ki * 72 + 65], 1.0)

            for p in range(4):
                nc.sync.dma_start(dbg_qp[b][p], qp[p][:])
            nc.sync.dma_start(dbg_qp[b][4], kdup[:])
            nc.sync.dma_start(dbg_qp[b][5][0:64, 0:576], va[:64, :])
            # ---------------- attention ----------------
            with tc.tile_pool(name="attps", bufs=1, space="PSUM") as aps, \
                    tc.tile_pool(name="scps", bufs=2, space="PSUM") as scps:
                ctxt_tiles = []
                for pr in range(4):
                    cps = [aps.tile([65, 1024], dt.float32, tag=f"ctx{j}", name=f"ctx{j}") for j in range(2)]
                    ets = []
                    for ki in range(8):
                        base = ki * 128
                        et = [exp_pool.tile([128, 1024], dt.bfloat16, tag=f"e{hh}_{ki}", name=f"e{hh}_{ki}")
                              for hh in range(2)]
                        nchunks = (1024 - base + 511) // 512
                        for cj in range(nchunks):
                            c0 = base + cj * 512
                            cw = min(512, 1024 - c0)
                            for hh in range(2):
                                sc = scps.tile([128, 512], dt.float32, tag="sc")
                                nc.tensor.matmul(
                                    sc[:, 0:cw],
                                    kdup[hh * 64:(hh + 1) * 64, base:base + 128],
                                    qp[pr][hh * 64:(hh + 1) * 64, c0:c0 + cw],
                                    start=True, stop=True,
                                )
                                if cj == 0:
                                    mc = (b * 8 + ki) * 128
                                    nc.vector.tensor_add(
                                        sc[:, 0:128], sc[:, 0:128],
                                        msk_sb[:, mc:mc + 128])
                                abc = b * 64 + ki * 8 + pr * 2 + hh
                                nc.scalar.activation(
                                    et[hh][:, c0:c0 + cw], sc[:, 0:cw], AF.Exp,
                                    bias=ab_sb[:, abc:abc + 1], scale=INV)
                        if pr == 0 and ki == 0:
                            nc.sync.dma_start(dbg_e[b], et[0][:])
                        ets.append(et)
                    # PSUM start=True clears has_written for the whole bank, so
                    # each region's accumulation group must run contiguously.
                    for qj in range(8):
                        qs = slice(qj * 128, qj * 128 + 128)
                        for ki in range(qj + 1):
                            for hh in range(2):
                                nc.tensor.matmul(
                                    cps[hh][:, qs],
                                    va[:, ki * 72: ki * 72 + 65],
                                    ets[ki][hh][:, qs],
                                    start=(ki == 0), stop=(ki == qj),
                                )
                    if pr == 0:
                        cps_sb = l_pool.tile([65, 1024], dt.float32, tag="cps_sb")
                        nc.scalar.copy(cps_sb[:], cps[0][:])
                        nc.sync.dma_start(dbg_cps[b], cps_sb[:])
                    # epilogue: normalize by the ones-row sums
                    ctxt = ctx_pool.tile([128, 1024], dt.bfloat16, tag=f"ctxt{pr}")
                    ctxt_tiles.append(ctxt)
                    for hh in range(2):
                        lsb = l_pool.tile([1, 1024], dt.float32, tag="lsb")
                        nc.scalar.copy(lsb[:], cps[hh][64:65, :])
                        lnl = l_pool.tile([1, 1024], dt.float32, tag="lnl")
                        nc.scalar.activation(lnl[:], lsb[:], AF.Ln)
                        rf = l_pool.tile([1, 1024], dt.float32, tag="rf")
                        nc.scalar.activation(rf[:], lnl[:], AF.Exp, scale=-1.0)
                        rr = l_pool.tile([1, 1024], dt.float32r, tag="rr")
                        nc.scalar.copy(rr[:], rf[:])
                        rb = rb_pool.tile([64, 1024], dt.float32, tag="rb")
                        for half in range(2):
                            hs_ = slice(half * 512, half * 512 + 512)
                            rps = scps.tile([64, 512], dt.float32, tag="rps")
                            nc.tensor.matmul(rps[:], ones_r[:], rr[:, hs_],
                                             start=True, stop=True)
                            nc.scalar.copy(rb[:, hs_], rps[:])
                        if pr == 0 and hh == 0:
                            nc.sync.dma_start(dbg_rb[b], rb[:])
                            nc.sync.dma_start(dbg_l[b][0:1], lsb[:])
                            nc.sync.dma_start(dbg_l[b][1:2], rf[:])
                        nc.vector.tensor_mul(
                            ctxt[hh * 64:(hh + 1) * 64, :], cps[hh][0:64, :], rb[:])

            for p in range(4):
                nc.sync.dma_start(dbg_ct[b][p], ctxt_tiles[p][:])
            # ---------------- dense partial -> DRAM ----------------
            with tc.tile_pool(name="dps", bufs=4, space="PSUM") as dpool, \
                    tc.tile_pool(name="dout", bufs=3) as dout_pool:
                for mt in range(32):
                    for n2 in range(2):
                        dps = dpool.tile([128, 512], dt.float32, tag="d")
                        for kt in range(4):
                            nc.tensor.matmul(
                                dps[:],
                                wd_sb[:, kt * 4096 + mt * 128: kt * 4096 + (mt + 1) * 128],
                                ctxt_tiles[kt][:, n2 * 512:(n2 + 1) * 512],
                                start=(kt == 0), stop=(kt == 3),
                            )
                        dsb = dout_pool.tile([128, 512], dt.float32, tag="dsb")
                        nc.scalar.copy(dsb[:], dps[:])
                        nc.sync.dma_start(
                            outp_d[mt][:, b * 1024 + n2 * 512: b * 1024 + n2 * 512 + 512],
                            dsb[:])

    _CACHED_NC = nc
    return nc


def host_prep(hidden_states, alibi, attention_mask, W_qkv, W_dense):
    hsT = np.ascontiguousarray(hidden_states.reshape(NPOS, HID).T).astype(bf16)
    hsT = hsT.reshape(32, 128, NPOS)

    j32 = np.arange(32)
    inv_freq = 1.0 / (10000.0 ** (2 * j32 / HD))
    t = np.arange(S, dtype=np.float64)
    fr = np.outer(inv_freq, t)                       # [32, S]
    cst = np.zeros((128, 2048), np.float32)
    cst[:, 0:1024] = np.tile(np.cos(fr), (4, 1))
    cst[:, 1024:2048] = np.tile(np.sin(fr), (4, 1))
    cst = cst.astype(bf16)

    mf = np.where(attention_mask[:, 0], -8e9, 0.0).astype(np.float32)  # [B,S,S]
    msk = np.zeros((128, 2048), np.float32)
    for b in range(2):
        for ki in range(8):
            blk = mf[b, ki * 128:(ki + 1) * 128, ki * 128:(ki + 1) * 128]
            msk[:, (b * 8 + ki) * 128:(b * 8 + ki + 1) * 128] = blk.T

    al = alibi.reshape(B, NKV * G, S) * INV          # [B, 64, S]

    perm = []
    for i in range(4):
        perm += [i * 64 + d for d in range(32)]
    for i in range(4):
        perm += [i * 64 + 32 + d for d in range(32)]
    for i in range(4, 8):
        perm += [i * 64 + d for d in range(32)]
    for i in range(4, 8):
        perm += [i * 64 + 32 + d for d in range(32)]
    perm += [512 + d for d in range(64)] + [576 + d for d in range(64)]
    perm = np.array(perm)

    idn = np.eye(128, dtype=np.float32).astype(bf16)
    in_maps = []
    for c in range(NCORES):
        Wg = W_qkv[c * 640:(c + 1) * 640][perm]       # [640, 4096]
        wq = np.ascontiguousarray(Wg.T).astype(bf16).reshape(32, 128, 640)
        Wd = W_dense[:, c * 512:(c + 1) * 512]        # [4096, 512]
        wd = np.ascontiguousarray(Wd.T).astype(bf16).reshape(4, 128, 4096)
        ab = np.zeros((128, 128), np.float32)
        for b in range(2):
            for ki in range(8):
                for h in range(8):
                    ab[:, b * 64 + ki * 8 + h] = al[b, c * 8 + h,
                                                    ki * 128:(ki + 1) * 128]
        in_maps.append({
            "hst": hsT, "wq": wq, "wd": wd, "cst": cst,
            "msk": msk, "ab": ab, "idn": idn,
        })
    return in_maps


def kernel(hidden_states, alibi, attention_mask, W_qkv, W_dense, _want_time=False):
    nc = build_program()
    in_maps = host_prep(np.asarray(hidden_states), np.asarray(alibi),
                        np.asarray(attention_mask), np.asarray(W_qkv),
                        np.asarray(W_dense))
    res = run_bass_kernel_spmd(nc, in_maps, list(range(NCORES)), trace=_want_time)
    acc = np.zeros((32, 128, NPOS), np.float32)
    for c in range(NCORES):
        acc += res.results[c]["outp"]
    out = acc.reshape(4096, NPOS).T.reshape(B, S, HID)
    if _want_time:
        return np.ascontiguousarray(out), res
    return np.ascontiguousarray(out)



# revision 14
# speedup vs baseline: 1.6236x; 1.6236x over previous
"""GQA attention block (dense_transformer) on 8 trn2 cores.

Sharding: tensor-parallel by kv-group. Core c owns kv-group c = 8 query
heads + 1 k + 1 v head (640 rows of W_qkv) and the matching 512 columns of
W_dense. hidden_states is replicated (passed transposed, bf16). Each core
returns a partial [4096, 2048] dense output in bf16; the host sums the 8
partials in f32.

On-core layout is feature-major ([feature, position]) throughout:
  qkvT = Wg @ hsT via PE (bf16), RoPE on DVE with a/b half-split weight
  permutation, scoresT[k,q] per k-tile (alibi enters as the per-partition
  ACT bias of the exp), PV with position-major V (PE transpose) augmented
  with a ones column so the softmax denominator falls out of the same
  matmul. Un-normalized context is evacuated early (bf16, error stays
  relative); the denominators for all 8 (pr,hh) pairs are batched into one
  [8,1024] Ln+Exp pass and broadcast across partitions with a one-hot
  selection matmul. PSUM->SBUF copies ride on DVE; ACT only does exp/Ln.
"""
import numpy as np
import ml_dtypes
from contextlib import ExitStack

import bass_rust
import concourse.bass as bass
import concourse.mybir as mybir
from concourse import tile
from concourse.bass_utils import run_bass_kernel_spmd
from concourse.vector_clock import ScopedClock

dt = mybir.dt
bf16 = ml_dtypes.bfloat16

B, S, HID = 2, 1024, 4096
NKV, G, HD = 8, 8, 64
NPOS = B * S
INV = 0.125
NCORES = 8

# ---------------------------------------------------------------------------
# walrus in this container takes at most ONE sync-wait per instruction; Tile
# attaches several (tail drain especially). Split extras onto same-engine nops.
_orig_exit = tile.TileContext.__exit__


def _split_waits(nc):
    for bb in nc.m.functions[0].blocks:
        out, extra = [], 0
        for inst in bb.instructions:
            si = inst.sync_info
            if si is not None and len(si.on_wait) > 1:
                waits = list(si.on_wait)
                for w in waits[:-1]:
                    nop = mybir.InstNoOp(name=f"I-wsplit-{nc.next_id()}")
                    nop.engine = inst.engine
                    nop.sync_info = bass_rust.SyncInfo(on_wait=[w], on_update=[])
                    nc.register_instruction(nop, overwrite=True)
                    out.append(nop)
                    extra += 1
                inst.sync_info = bass_rust.SyncInfo(
                    on_wait=[waits[-1]], on_update=list(si.on_update)
                )
            out.append(inst)
        if extra:
            bb.instructions = out


def _patched_exit(self, exc_type, exc_val, exc_tb):
    r = _orig_exit(self, exc_type, exc_val, exc_tb)
    _split_waits(self.nc)
    return r


tile.TileContext.__exit__ = _patched_exit
# ---------------------------------------------------------------------------

_CACHED_NC = None


def build_program():
    global _CACHED_NC
    if _CACHED_NC is not None:
        return _CACHED_NC
    nc = bass.Bass()
    hst_d = nc.declare_dram_parameter("hst", [32, 128, NPOS], dt.bfloat16, isOutput=False)
    wq_d = nc.declare_dram_parameter("wq", [32, 128, 640], dt.bfloat16, isOutput=False)
    wd_d = nc.declare_dram_parameter("wd", [4, 128, 4096], dt.bfloat16, isOutput=False)
    cst_d = nc.declare_dram_parameter("cst", [128, 2048], dt.bfloat16, isOutput=False)
    msk_d = nc.declare_dram_parameter("msk", [128, 2048], dt.float32, isOutput=False)
    ab_d = nc.declare_dram_parameter("ab", [128, 128], dt.float32, isOutput=False)
    idn_d = nc.declare_dram_parameter("idn", [128, 128], dt.bfloat16, isOutput=False)
    sel_d = nc.declare_dram_parameter("sel", [8, 512], dt.bfloat16, isOutput=False)
    outp_d = nc.declare_dram_parameter("outp", [32, 128, NPOS], dt.bfloat16, isOutput=True)

    AF = mybir.ActivationFunctionType

    with ExitStack() as ctx:
        tc = ctx.enter_context(tile.TileContext(nc))
        cpool = ctx.enter_context(tc.tile_pool(name="const", bufs=1))
        wq_sb = cpool.tile([128, 32 * 640], dt.bfloat16)
        for k in range(32):
            nc.sync.dma_start(wq_sb[:, k * 640:(k + 1) * 640], wq_d[k])
        wd_sb = cpool.tile([128, 4 * 4096], dt.bfloat16)
        for kt in range(4):
            nc.sync.dma_start(wd_sb[:, kt * 4096:(kt + 1) * 4096], wd_d[kt])
        cst_sb = cpool.tile([128, 2048], dt.bfloat16)
        nc.sync.dma_start(cst_sb[:], cst_d[:])
        msk_sb = cpool.tile([128, 2048], dt.float32)
        nc.sync.dma_start(msk_sb[:], msk_d[:])
        ab_sb = cpool.tile([128, 128], dt.float32)
        nc.sync.dma_start(ab_sb[:], ab_d[:])
        idn_sb = cpool.tile([128, 128], dt.bfloat16)
        nc.sync.dma_start(idn_sb[:], idn_d[:])
        sel_sb = cpool.tile([8, 512], dt.bfloat16)
        nc.sync.dma_start(sel_sb[:], sel_d[:])

        hs_pool = ctx.enter_context(tc.tile_pool(name="hs", bufs=4))
        raw_pool = ctx.enter_context(tc.tile_pool(name="raw", bufs=2))
        tmp_pool = ctx.enter_context(tc.tile_pool(name="tmp", bufs=2))
        pk_pool = ctx.enter_context(tc.tile_pool(name="pk", bufs=2))
        qp_pool = ctx.enter_context(tc.tile_pool(name="qp", bufs=1))
        kv_pool = ctx.enter_context(tc.tile_pool(name="kv", bufs=1))
        va_pool = ctx.enter_context(tc.tile_pool(name="va", bufs=1))
        exp_pool = ctx.enter_context(tc.tile_pool(name="exp", bufs=1))
        l_pool = ctx.enter_context(tc.tile_pool(name="l", bufs=1))
        rb_pool = ctx.enter_context(tc.tile_pool(name="rb", bufs=2))
        ctx_pool = ctx.enter_context(tc.tile_pool(name="ctx", bufs=1))

        for b in range(2):
            # ---------------- QKV projection + RoPE ----------------
            qp = [qp_pool.tile([128, 1024], dt.bfloat16, tag=f"qp{p}", name=f"qp{p}") for p in range(4)]
            kdup = kv_pool.tile([128, 1024], dt.bfloat16, tag="kdup")
            vt = kv_pool.tile([64, 1024], dt.bfloat16, tag="vt")
            with tc.tile_pool(name="qkvps", bufs=1, space="PSUM") as qkv_psum:
                for n in range(2):
                    pcol = b * 1024 + n * 512
                    ncol = slice(n * 512, n * 512 + 512)
                    ps = [qkv_psum.tile([128, 512], dt.float32, tag=f"qkv{m}", name=f"qkv{m}")
                          for m in range(5)]
                    for k in range(32):
                        hs_t = hs_pool.tile([128, 512], dt.bfloat16)
                        nc.sync.dma_start(hs_t[:], hst_d[k][:, pcol:pcol + 512])
                        for m in range(5):
                            nc.tensor.matmul(
                                ps[m][:],
                                wq_sb[:, k * 640 + m * 128: k * 640 + (m + 1) * 128],
                                hs_t[:],
                                start=(k == 0), stop=(k == 31),
                            )
                    raw = [raw_pool.tile([128, 512], dt.bfloat16, tag=f"raw{m}", name=f"raw{m}")
                           for m in range(5)]
                    for m in range(5):
                        nc.vector.tensor_copy(raw[m][:], ps[m][:])
                    Cs = cst_sb[:, n * 512:(n + 1) * 512]
                    Ss = cst_sb[:, 1024 + n * 512: 1024 + (n + 1) * 512]
                    for grp in range(2):
                        A, Bb = raw[grp * 2], raw[grp * 2 + 1]
                        P1 = tmp_pool.tile([128, 512], dt.bfloat16, tag="P1")
                        P2 = tmp_pool.tile([128, 512], dt.bfloat16, tag="P2")
                        P3 = tmp_pool.tile([128, 512], dt.bfloat16, tag="P3")
                        P4 = tmp_pool.tile([128, 512], dt.bfloat16, tag="P4")
                        nc.vector.tensor_mul(P1[:], A[:], Cs)
                        nc.vector.tensor_mul(P2[:], Bb[:], Ss)
                        nc.vector.tensor_mul(P3[:], Bb[:], Cs)
                        nc.vector.tensor_mul(P4[:], A[:], Ss)
                        for i in range(4):
                            h = grp * 4 + i
                            pr, sub = h // 2, h % 2
                            sl = slice(32 * i, 32 * i + 32)
                            nc.vector.tensor_sub(
                                qp[pr][sub * 64: sub * 64 + 32, ncol], P1[sl, :], P2[sl, :])
                            nc.vector.tensor_add(
                                qp[pr][sub * 64 + 32: sub * 64 + 64, ncol], P3[sl, :], P4[sl, :])
                    kvr = raw[4]
                    pk1 = pk_pool.tile([32, 512], dt.bfloat16, tag="pk1")
                    pk2 = pk_pool.tile([32, 512], dt.bfloat16, tag="pk2")
                    pk3 = pk_pool.tile([32, 512], dt.bfloat16, tag="pk3")
                    pk4 = pk_pool.tile([32, 512], dt.bfloat16, tag="pk4")
                    nc.vector.tensor_mul(pk1[:], kvr[0:32, :], Cs[0:32, :])
                    nc.vector.tensor_mul(pk2[:], kvr[32:64, :], Ss[32:64, :])
                    nc.vector.tensor_mul(pk3[:], kvr[32:64, :], Cs[32:64, :])
                    nc.vector.tensor_mul(pk4[:], kvr[0:32, :], Ss[0:32, :])
                    nc.vector.tensor_sub(kdup[0:32, ncol], pk1[:], pk2[:])
                    nc.vector.tensor_add(kdup[32:64, ncol], pk3[:], pk4[:])
                    nc.vector.tensor_copy(kdup[64:128, ncol], kdup[0:64, ncol])
                    nc.vector.tensor_copy(vt[:, ncol], kvr[64:128, :])

            # ---------------- V transpose + ones column ----------------
            va = va_pool.tile([128, 8 * 72], dt.bfloat16, tag="va")
            with tc.tile_pool(name="vtps", bufs=2, space="PSUM") as vt_psum:
                for ki in range(8):
                    vps = vt_psum.tile([128, 64], dt.bfloat16, tag="vps")
                    nc.tensor.transpose(vps[:], vt[0:64, ki * 128:(ki + 1) * 128],
                                        idn_sb[0:64, 0:64])
                    nc.scalar.copy(va[:, ki * 72: ki * 72 + 64], vps[:])
                    nc.vector.memset(va[:, ki * 72 + 64: ki * 72 + 65], 1.0)

            # ---------------- attention ----------------
            L8 = l_pool.tile([8, 1024], dt.bfloat16, tag="L8")
            ctxu_tiles = []
            with tc.tile_pool(name="attps", bufs=1, space="PSUM") as aps, \
                    tc.tile_pool(name="scps", bufs=2, space="PSUM") as scps:
                for pr in range(4):
                    cps = [aps.tile([65, 1024], dt.float32, tag=f"ctx{j}", name=f"ctx{j}") for j in range(2)]
                    ets = []
                    for ki in range(8):
                        base = ki * 128
                        et = [exp_pool.tile([128, 1024], dt.bfloat16, tag=f"e{hh}_{ki}", name=f"e{hh}_{ki}")
                              for hh in range(2)]
                        nchunks = (1024 - base + 511) // 512
                        for cj in range(nchunks):
                            c0 = base + cj * 512
                            cw = min(512, 1024 - c0)
                            for hh in range(2):
                                sc = scps.tile([128, 512], dt.float32, tag="sc")
                                nc.tensor.matmul(
                                    sc[:, 0:cw],
                                    kdup[hh * 64:(hh + 1) * 64, base:base + 128],
                                    qp[pr][hh * 64:(hh + 1) * 64, c0:c0 + cw],
                                    start=True, stop=True,
                                )
                                if cj == 0:
                                    mc = (b * 8 + ki) * 128
                                    nc.vector.tensor_add(
                                        sc[:, 0:128], sc[:, 0:128],
                                        msk_sb[:, mc:mc + 128])
                                abc = b * 64 + ki * 8 + pr * 2 + hh
                                nc.scalar.activation(
                                    et[hh][:, c0:c0 + cw], sc[:, 0:cw], AF.Exp,
                                    bias=ab_sb[:, abc:abc + 1], scale=INV)
                        ets.append(et)
                    # PSUM start=True clears has_written for the whole bank, so
                    # each region's accumulation group must run contiguously.
                    for qj in range(8):
                        qs = slice(qj * 128, qj * 128 + 128)
                        for ki in range(qj + 1):
                            for hh in range(2):
                                nc.tensor.matmul(
                                    cps[hh][:, qs],
                                    va[:, ki * 72: ki * 72 + 65],
                                    ets[ki][hh][:, qs],
                                    start=(ki == 0), stop=(ki == qj),
                                )
                    # evacuate un-normalized context + denominators (DVE)
                    ctxu = ctx_pool.tile([128, 1024], dt.bfloat16, tag=f"ctxu{pr}")
                    ctxu_tiles.append(ctxu)
                    for hh in range(2):
                        nc.vector.tensor_copy(
                            ctxu[hh * 64:(hh + 1) * 64, :], cps[hh][0:64, :])
                        # engines can't shift partition start mod 32; bounce the
                        # denominator row through a p0 tile + SBUF->SBUF DMA.
                        lrow = rb_pool.tile([1, 1024], dt.bfloat16, tag="lrow")
                        nc.vector.tensor_copy(lrow[:], cps[hh][64:65, :])
                        idx = pr * 2 + hh
                        nc.sync.dma_start(L8[idx:idx + 1, :], lrow[:])

                # batched denominator: r = 1/l for all 8 rows at once
                LL = l_pool.tile([8, 1024], dt.float32, tag="LL")
                nc.scalar.activation(LL[:], L8[:], AF.Ln)
                RR = l_pool.tile([8, 1024], dt.bfloat16, tag="RR")
                nc.scalar.activation(RR[:], LL[:], AF.Exp, scale=-1.0)
                # broadcast each r row across 64 partitions and normalize
                ctxt_tiles = []
                for pr in range(4):
                    ctxt = ctx_pool.tile([128, 1024], dt.bfloat16, tag=f"ctxt{pr}")
                    ctxt_tiles.append(ctxt)
                    rbs = rb_pool.tile([128, 1024], dt.bfloat16, tag="rbs")
                    for half in range(2):
                        hs_ = slice(half * 512, half * 512 + 512)
                        rps = scps.tile([128, 512], dt.float32, tag="rps")
                        for hh in range(2):
                            idx = pr * 2 + hh
                            nc.tensor.matmul(rps[hh * 64:(hh + 1) * 64, :],
                                             sel_sb[:, idx * 64:(idx + 1) * 64],
                                             RR[:, hs_], start=True, stop=True)
                        nc.vector.tensor_copy(rbs[:, hs_], rps[:])
                    nc.vector.tensor_mul(ctxt[:], ctxu_tiles[pr][:], rbs[:])

            # ---------------- dense partial -> DRAM ----------------
            with tc.tile_pool(name="dps", bufs=4, space="PSUM") as dpool, \
                    tc.tile_pool(name="dout", bufs=3) as dout_pool:
                for mt in range(32):
                    for n2 in range(2):
                        dps = dpool.tile([128, 512], dt.float32, tag="d")
                        for kt in range(4):
                            nc.tensor.matmul(
                                dps[:],
                                wd_sb[:, kt * 4096 + mt * 128: kt * 4096 + (mt + 1) * 128],
                                ctxt_tiles[kt][:, n2 * 512:(n2 + 1) * 512],
                                start=(kt == 0), stop=(kt == 3),
                            )
                        dsb = dout_pool.tile([128, 512], dt.bfloat16, tag="dsb")
                        nc.vector.tensor_copy(dsb[:], dps[:])
                        nc.sync.dma_start(
                            outp_d[mt][:, b * 1024 + n2 * 512: b * 1024 + n2 * 512 + 512],
                            dsb[:])

    _CACHED_NC = nc
    return nc


def host_prep(hidden_states, alibi, attention_mask, W_qkv, W_dense):
    hsT = np.ascontiguousarray(hidden_states.reshape(NPOS, HID).T).astype(bf16)
    hsT = hsT.reshape(32, 128, NPOS)

    j32 = np.arange(32)
    inv_freq = 1.0 / (10000.0 ** (2 * j32 / HD))
    t = np.arange(S, dtype=np.float64)
    fr = np.outer(inv_freq, t)                       # [32, S]
    cst = np.zeros((128, 2048), np.float32)
    cst[:, 0:1024] = np.tile(np.cos(fr), (4, 1))
    cst[:, 1024:2048] = np.tile(np.sin(fr), (4, 1))
    cst = cst.astype(bf16)

    mf = np.where(attention_mask[:, 0], -8e9, 0.0).astype(np.float32)  # [B,S,S]
    msk = np.zeros((128, 2048), np.float32)
    for b in range(2):
        for ki in range(8):
            blk = mf[b, ki * 128:(ki + 1) * 128, ki * 128:(ki + 1) * 128]
            msk[:, (b * 8 + ki) * 128:(b * 8 + ki + 1) * 128] = blk.T

    al = alibi.reshape(B, NKV * G, S) * INV          # [B, 64, S]

    perm = []
    for i in range(4):
        perm += [i * 64 + d for d in range(32)]
    for i in range(4):
        perm += [i * 64 + 32 + d for d in range(32)]
    for i in range(4, 8):
        perm += [i * 64 + d for d in range(32)]
    for i in range(4, 8):
        perm += [i * 64 + 32 + d for d in range(32)]
    perm += [512 + d for d in range(64)] + [576 + d for d in range(64)]
    perm = np.array(perm)

    idn = np.eye(128, dtype=np.float32).astype(bf16)
    sel = np.kron(np.eye(8, dtype=np.float32), np.ones((1, 64), np.float32)).astype(bf16)
    in_maps = []
    for c in range(NCORES):
        Wg = W_qkv[c * 640:(c + 1) * 640][perm]       # [640, 4096]
        wq = np.ascontiguousarray(Wg.T).astype(bf16).reshape(32, 128, 640)
        Wd = W_dense[:, c * 512:(c + 1) * 512]        # [4096, 512]
        wd = np.ascontiguousarray(Wd.T).astype(bf16).reshape(4, 128, 4096)
        ab = np.zeros((128, 128), np.float32)
        for b in range(2):
            for ki in range(8):
                for h in range(8):
                    ab[:, b * 64 + ki * 8 + h] = al[b, c * 8 + h,
                                                    ki * 128:(ki + 1) * 128]
        in_maps.append({
            "hst": hsT, "wq": wq, "wd": wd, "cst": cst,
            "msk": msk, "ab": ab, "idn": idn, "sel": sel,
        })
    return in_maps


def kernel(hidden_states, alibi, attention_mask, W_qkv, W_dense, _want_time=False):
    nc = build_program()
    in_maps = host_prep(np.asarray(hidden_states), np.asarray(alibi),
                        np.asarray(attention_mask), np.asarray(W_qkv),
                        np.asarray(W_dense))
    res = run_bass_kernel_spmd(nc, in_maps, list(range(NCORES)), trace=_want_time)
    acc = np.zeros((32, 128, NPOS), np.float32)
    for c in range(NCORES):
        acc += res.results[c]["outp"].astype(np.float32)
    out = acc.reshape(4096, NPOS).T.reshape(B, S, HID)
    if _want_time:
        return np.ascontiguousarray(out), res
    return np.ascontiguousarray(out)
